# revision 1
# baseline (speedup 1.0000x reference)
"""AdditiveAttention Trainium2 kernel (8 NeuronCores, data-parallel over batch).

Reference computation (B=32, T=2048, D=U=512, fp32):
    query = values[:, -1] @ W2_w + W2_b                     # [B, U]
    keys  = values @ W1_w + W1_b                            # [B, T, U]
    score = tanh(keys + query[:, None, :]) @ V_w + V_b      # [B, T, 1]
    attn  = softmax(score, axis=1)
    out   = sum(attn * values, axis=1)                      # [B, D]

Sharding: data-parallel over B (4 batches per core), weights replicated,
no collectives.  Compute in bf16 on the TensorEngine (fp32 accumulate in
PSUM); validated end-to-end rel-err ~3e-3 vs the fp32 reference.

Layout/scheduling notes (from perfetto traces):
  - the xbar serializes on every DMA transpose<->copy mode transition, so
    ALL 16 values-transpose DMAs run back-to-back up-front (sync queue);
    copies (weights via gpsimd before, nat/e4/out after) never interleave
  - last rows for the query come from one natural DMA + PE transposes
  - keysT accumulates into a 2-bank PSUM tile ([128, 1024], two 512-chunk
    halves) so one tanh serves two T-chunks (halves ACT op count)
  - the 4 score matmuls per u-chunk are col-tiled (tile_position) across PE
    column groups -> concurrent, out strips at partitions 0/32/64/96
  - exp reads the score strips straight from PSUM; Z and 1/Z on DVE;
    e stays unnormalized, 1/Z folds into the output copy
  - weighted sum col-tiles the 4 batches across PE column groups
V_b drops out of softmax (constant shift).
"""

from contextlib import ExitStack

import numpy as np
import ml_dtypes

import concourse.bass as bass
import concourse.tile as tile
from concourse import bacc, mybir
from concourse.bass_utils import run_bass_kernel_spmd

BF16 = ml_dtypes.bfloat16

B, T, D, U = 32, 2048, 512, 512
NCORES = 8
BSH = B // NCORES          # 4 batches per core
P = 128
DC = D // P                # 4 chunks of D
UC = U // P                # 4 chunks of U
TS = 512                   # T tile (score chunk)
TN = T // TS               # 4
SP2 = 2 * TS               # paired T tile for keys/tanh (2 PSUM banks)
NPAIR = T // SP2           # 2
TK = T // P                # 16 chunks of T for transposes / weighted sum

_GRAPH = None


def _build_graph():
    nc = bacc.Bacc("TRN2", target_bir_lowering=False, debug=False)
    bf = mybir.dt.bfloat16
    f32 = mybir.dt.float32

    vals = nc.declare_dram_parameter("vals", [BSH, T, D], bf, isOutput=False)
    w1 = nc.declare_dram_parameter("w1", [D, U], bf, isOutput=False)
    w2 = nc.declare_dram_parameter("w2", [D, U], bf, isOutput=False)
    vw = nc.declare_dram_parameter("vw", [U, 1], bf, isOutput=False)
    bsum = nc.declare_dram_parameter("bsum", [U, 1], f32, isOutput=False)
    ident = nc.declare_dram_parameter("ident", [BSH, BSH], bf, isOutput=False)
    out_ext = nc.declare_dram_parameter("out", [BSH, D], f32, isOutput=True)

    Tanh = mybir.ActivationFunctionType.Tanh
    Exp = mybir.ActivationFunctionType.Exp

    with tile.TileContext(nc) as tc, ExitStack() as ctx:
        const = ctx.enter_context(tc.tile_pool(name="const", bufs=1))
        valt_pool = ctx.enter_context(tc.tile_pool(name="valt", bufs=BSH))
        nat_pool = ctx.enter_context(tc.tile_pool(name="nat", bufs=BSH))
        tk_pool = ctx.enter_context(tc.tile_pool(name="tk", bufs=3))
        sm_pool = ctx.enter_context(tc.tile_pool(name="sm", bufs=1))
        kps = ctx.enter_context(tc.tile_pool(name="kps", bufs=2, space="PSUM"))
        sps = ctx.enter_context(tc.tile_pool(name="sps", bufs=2, space="PSUM"))
        aps = ctx.enter_context(tc.tile_pool(name="aps", bufs=2, space="PSUM"))

        # ---- everything on ONE queue (sync) in strict copy -> transpose ->
        # copy order: the xbar serializes on every transpose<->copy mode
        # transition, so the stream must not alternate
        lastrows = const.tile([BSH, D], bf)
        nc.sync.dma_start(lastrows[:], vals.ap()[:, T - 1, :])
        w2_sb = const.tile([P, DC, U], bf)
        nc.sync.dma_start(w2_sb[:], w2.ap().rearrange("(c p) u -> p c u", p=P))
        ident_sb = const.tile([BSH, BSH], bf)
        nc.sync.dma_start(ident_sb[:], ident.ap())
        w1_sb = const.tile([P, DC, U], bf)
        nc.sync.dma_start(w1_sb[:], w1.ap().rearrange("(c p) u -> p c u", p=P))
        bsum_sb = const.tile([P, UC], f32)
        nc.sync.dma_start(
            bsum_sb[:], bsum.ap().rearrange("(c p) one -> p (c one)", p=P)
        )
        v_sb = const.tile([P, UC], bf)
        nc.sync.dma_start(v_sb[:], vw.ap().rearrange("(c p) one -> p (c one)", p=P))

        # ---- ALL values-transpose DMAs back-to-back (one xbar mode run)
        valts = []
        for b in range(BSH):
            valt = valt_pool.tile([P, DC, T], bf, tag="valt")
            if b <= 1:
                for h in range(2):
                    for c in range(DC):
                        nc.sync.dma_start(
                            valt[:, c, h * (T // 2) : (h + 1) * (T // 2)],
                            vals.ap()[
                                b, h * (T // 2) : (h + 1) * (T // 2),
                                c * P : (c + 1) * P,
                            ],
                            transpose=True,
                        )
            else:
                for c in range(DC):
                    nc.sync.dma_start(
                        valt[:, c], vals.ap()[b, :, c * P : (c + 1) * P],
                        transpose=True,
                    )
            valts.append(valt)

        # natural-layout loads for the weighted sum (sync FIFO puts them
        # after all transposes; needed only at the tail)
        nats = []
        for b in range(BSH):
            nat_b = nat_pool.tile([P, TK, D], bf, tag="nat")
            nc.sync.dma_start(
                nat_b[:], vals.ap()[b].rearrange("(n p) d -> p n d", p=P)
            )
            nats.append(nat_b)

        # ---- last rows -> lastT via PE transpose; query for all batches
        lastT = const.tile([P, DC, BSH], bf)
        for c in range(DC):
            lp = aps.tile([P, BSH], bf, tag="aps")
            nc.tensor.transpose(
                lp[:], lastrows[:, c * P : (c + 1) * P], ident_sb[:]
            )
            nc.vector.tensor_copy(lastT[:, c, :], lp[:])

        qb = const.tile([P, UC, BSH], f32)
        for u in range(UC):
            qp = aps.tile([P, BSH], f32, tag="aps")
            for c in range(DC):
                nc.tensor.matmul(
                    qp[:],
                    w2_sb[:, c, u * P : (u + 1) * P],
                    lastT[:, c, :],
                    start=(c == 0),
                    stop=(c == DC - 1),
                )
            nc.vector.tensor_scalar_add(qb[:, u], qp[:], bsum_sb[:, u : u + 1])

        # per-batch softmax state (partition 0; col 0 = Z, col 1 = 1/Z)
        e_rows = [
            sm_pool.tile([1, T], bf, name=f"erow{b}", tag=f"erow{b}")
            for b in range(BSH)
        ]
        zr = [
            sm_pool.tile([1, 2], f32, name=f"zr{b}", tag=f"zr{b}")
            for b in range(BSH)
        ]
        zparts = [
            sm_pool.tile([1, TN], f32, name=f"zp{b}", tag=f"zp{b}")
            for b in range(BSH)
        ]
        e4 = sm_pool.tile([BSH, T], bf)

        # ---- main phase: keys -> tanh -> score, batch-major ------------
        for b in range(BSH):
            valt = valts[b]
            # score strips: chunk s lives at partition 32*s of one PSUM tile
            scp = sps.tile([P, TS], f32, tag="sps")
            for u in range(UC):
                tkts = []
                for pair in range(NPAIR):
                    kp = kps.tile([P, SP2], f32, tag="kps")
                    for half in range(2):
                        s0 = pair * SP2 + half * TS
                        for c in range(DC):
                            nc.tensor.matmul(
                                kp[:, half * TS : (half + 1) * TS],
                                w1_sb[:, c, u * P : (u + 1) * P],
                                valt[:, c, s0 : s0 + TS],
                                start=(c == 0),
                                stop=(c == DC - 1),
                            )
                    tkt = tk_pool.tile([P, SP2], bf, tag="tk")
                    nc.scalar.activation(
                        tkt[:], kp[:], Tanh, bias=qb[:, u, b : b + 1]
                    )
                    tkts.append(tkt)
                # 4 score matmuls col-tiled across PE column groups
                for s in range(TN):
                    nc.tensor.matmul(
                        scp[32 * s : 32 * s + 1, :],
                        v_sb[:, u : u + 1],
                        tkts[s // 2][:, (s % 2) * TS : (s % 2 + 1) * TS],
                        start=(u == 0),
                        stop=(u == UC - 1),
                        tile_position=(0, 32 * s),
                        skip_group_check=True,
                    )
            # exp straight from the PSUM strips; per-chunk Z partials and
            # per-chunk e4 assembly keep batch 3's tail chain short
            zp = zparts[b]
            for s in range(TN):
                nc.scalar.activation(
                    e_rows[b][0:1, s * TS : (s + 1) * TS],
                    scp[32 * s : 32 * s + 1, :],
                    Exp,
                )
                nc.vector.tensor_reduce(
                    zp[:, s : s + 1],
                    e_rows[b][0:1, s * TS : (s + 1) * TS],
                    mybir.AxisListType.X, mybir.AluOpType.add,
                )
                nc.sync.dma_start(
                    e4[b : b + 1, s * TS : (s + 1) * TS],
                    e_rows[b][0:1, s * TS : (s + 1) * TS],
                )
            nc.vector.tensor_reduce(
                zr[b][:, 0:1], zp[:], mybir.AxisListType.X, mybir.AluOpType.add,
            )
            nc.vector.reciprocal(zr[b][:, 1:2], zr[b][:, 0:1])

        # ---- tail: transpose e chunks + col-tiled weighted sum ----------
        wp = sps.tile([P, D], f32, tag="sps")
        at_sb = sm_pool.tile([P, TK, BSH], bf)
        for k in range(TK):
            ap_t = aps.tile([P, BSH], bf, tag="aps")
            nc.tensor.transpose(
                ap_t[:], e4[:, k * P : (k + 1) * P], ident_sb[:]
            )
            nc.vector.tensor_copy(at_sb[:, k, :], ap_t[:])
            for b in range(BSH):
                nc.tensor.matmul(
                    wp[32 * b : 32 * b + 1, :],
                    at_sb[:, k, b : b + 1],
                    nats[b][:, k],
                    start=(k == 0),
                    stop=(k == TK - 1),
                    tile_position=(0, 32 * b),
                    skip_group_check=True,
                )
        for b in range(BSH):
            ob = sm_pool.tile([1, D], f32, name=f"ob{b}", tag=f"ob{b}")
            nc.vector.tensor_scalar_mul(
                ob[:], wp[32 * b : 32 * b + 1, :], zr[b][:, 1:2]
            )
            nc.sync.dma_start(out_ext.ap()[b : b + 1, :], ob[:])

    nc.finalize()
    return nc


def _get_graph():
    global _GRAPH
    if _GRAPH is None:
        _GRAPH = _build_graph()
    return _GRAPH


def _make_in_maps(values, W1_w, W1_b, W2_w, W2_b, V_w, V_b):
    vals_bf = np.ascontiguousarray(values).astype(BF16)
    w1_bf = np.ascontiguousarray(W1_w).astype(BF16)
    w2_bf = np.ascontiguousarray(W2_w).astype(BF16)
    v_bf = np.ascontiguousarray(V_w).astype(BF16)
    bsum = (
        np.asarray(W1_b, np.float32) + np.asarray(W2_b, np.float32)
    ).reshape(U, 1)
    ident = np.eye(BSH, dtype=BF16)

    in_maps = []
    for core in range(NCORES):
        sl = slice(core * BSH, (core + 1) * BSH)
        in_maps.append(
            {
                "vals": vals_bf[sl],
                "w1": w1_bf,
                "w2": w2_bf,
                "vw": v_bf,
                "bsum": bsum,
                "ident": ident,
            }
        )
    return in_maps


def run(inputs, trace=False, **kw):
    """Build + run on 8 cores; returns (full_output, BassKernelResults)."""
    nc = _get_graph()
    in_maps = _make_in_maps(**inputs)
    res = run_bass_kernel_spmd(
        nc, in_maps, core_ids=list(range(NCORES)), trace=trace, **kw
    )
    out = np.concatenate([np.asarray(r["out"]) for r in res.results], axis=0)
    return out.astype(np.float32), res


def kernel(**inputs) -> np.ndarray:
    out, _ = run(inputs)
    return out



# revision 5
# speedup vs baseline: 1.0236x; 1.0236x over previous
"""AdditiveAttention Trainium2 kernel (8 NeuronCores, data-parallel over batch).

Reference computation (B=32, T=2048, D=U=512, fp32):
    query = values[:, -1] @ W2_w + W2_b                     # [B, U]
    keys  = values @ W1_w + W1_b                            # [B, T, U]
    score = tanh(keys + query[:, None, :]) @ V_w + V_b      # [B, T, 1]
    attn  = softmax(score, axis=1)
    out   = sum(attn * values, axis=1)                      # [B, D]

Sharding: data-parallel over B (4 batches per core), weights replicated,
no collectives.  bf16 matmuls (fp32 PSUM accumulate); rel-err ~3e-3.

v2 design (vs the DMA-transpose baseline):
  - values are pre-transposed on the HOST into [b, r, c, t] (valt) and
    pre-tiled natural [b, r, k, d] (nat): every DMA is a plain copy with
    16KB-contiguous per-partition lines (no xbar transpose mode, ~128
    descriptors per tensor instead of ~45K 1KB packets)
  - per-batch pipeline: keys (PE) -> tanh (ACT, bias=query) -> score
    strips col-tiled by T-chunk at PSUM partitions 32s (PE column
    groups run concurrently) -> one strided-partition Exp per batch
  - e rows are assembled into e4 [4, T] by a single SBUF->SBUF DMA per
    batch on the vector queue (keeps the sync FIFO for bulk loads)
  - tail: 16 PE transposes of e4 chunks + wsum matmuls col-tiled by
    batch; Z = sum(e) comes from a ones-stationary matmul per batch so
    softmax normalization lands at partition 32b and folds into one
    scalar-engine Copy(scale=1/Z); no DVE reductions over [1, T]
"""

from contextlib import ExitStack

import numpy as np
import ml_dtypes

import concourse.bass as bass
import concourse.tile as tile
from concourse import bacc, mybir
from concourse.bass_utils import run_bass_kernel_spmd

BF16 = ml_dtypes.bfloat16

B, T, D, U = 32, 2048, 512, 512
NCORES = 8
BSH = B // NCORES          # 4 batches per core
P = 128
DC = D // P                # 4 chunks of D
UC = U // P                # 4 chunks of U
TS = 512                   # score strip / T tile
TN = T // TS               # 4 strips
TK = T // P                # 16 chunks of T for the weighted sum

_GRAPH = None


def _build_graph():
    nc = bacc.Bacc("TRN2", target_bir_lowering=False, debug=False)
    bf = mybir.dt.bfloat16
    f32 = mybir.dt.float32

    valt = nc.declare_dram_parameter("valt", [BSH, P, DC, T], bf, isOutput=False)
    nat = nc.declare_dram_parameter("nat", [BSH, P, TK, D], bf, isOutput=False)
    w1 = nc.declare_dram_parameter("w1", [P, DC, U], bf, isOutput=False)
    w2 = nc.declare_dram_parameter("w2", [P, DC, U], bf, isOutput=False)
    vw = nc.declare_dram_parameter("vw", [P, UC], bf, isOutput=False)
    bsum = nc.declare_dram_parameter("bsum", [P, UC], f32, isOutput=False)
    ones = nc.declare_dram_parameter("ones", [P, 1], bf, isOutput=False)
    lastrows = nc.declare_dram_parameter("lastrows", [BSH, D], bf, isOutput=False)
    ident = nc.declare_dram_parameter("ident", [BSH, BSH], bf, isOutput=False)
    out_ext = nc.declare_dram_parameter("out", [BSH, D], f32, isOutput=True)

    Tanh = mybir.ActivationFunctionType.Tanh
    Exp = mybir.ActivationFunctionType.Exp

    with tile.TileContext(nc) as tc, ExitStack() as ctx:
        const = ctx.enter_context(tc.tile_pool(name="const", bufs=1))
        valt_pool = ctx.enter_context(tc.tile_pool(name="valt", bufs=BSH))
        nat_pool = ctx.enter_context(tc.tile_pool(name="nat", bufs=BSH))
        tk_pool = ctx.enter_context(tc.tile_pool(name="tk", bufs=3))
        kps = ctx.enter_context(tc.tile_pool(name="kps", bufs=2, space="PSUM"))
        sps = ctx.enter_context(tc.tile_pool(name="sps", bufs=2, space="PSUM"))
        aps = ctx.enter_context(tc.tile_pool(name="aps", bufs=2, space="PSUM"))

        # ---- bulk loads: plain copies, big contiguous descriptors ------
        ident_sb = const.tile([BSH, BSH], bf)
        nc.sync.dma_start(ident_sb[:], ident.ap())
        lastrows_sb = const.tile([BSH, D], bf)
        nc.sync.dma_start(lastrows_sb[:], lastrows.ap())
        bsum_sb = const.tile([P, UC], f32)
        nc.sync.dma_start(bsum_sb[:], bsum.ap())
        v_sb = const.tile([P, UC], bf)
        nc.sync.dma_start(v_sb[:], vw.ap())
        ones_sb = const.tile([P, 1], bf)
        nc.sync.dma_start(ones_sb[:], ones.ap())
        w2_sb = const.tile([P, DC, U], bf)
        nc.sync.dma_start(w2_sb[:], w2.ap())
        w1_sb = const.tile([P, DC, U], bf)
        nc.sync.dma_start(w1_sb[:], w1.ap())

        valts = []
        for b in range(BSH):
            valt_b = valt_pool.tile([P, DC, T], bf, tag="valt")
            if b == 0:
                # split the first batch so keys(b0, u0) can start sooner
                nc.sync.dma_start(
                    valt_b[:, :, 0 : T // 2], valt.ap()[b, :, :, 0 : T // 2]
                )
                nc.sync.dma_start(
                    valt_b[:, :, T // 2 : T], valt.ap()[b, :, :, T // 2 : T]
                )
            else:
                nc.sync.dma_start(valt_b[:], valt.ap()[b])
            valts.append(valt_b)
        nats = []
        for b in range(BSH):
            nat_b = nat_pool.tile([P, TK, D], bf, tag="nat")
            nc.sync.dma_start(nat_b[:], nat.ap()[b])
            nats.append(nat_b)

        # ---- query: lastrows -> lastT via PE transpose; q = lastT.T@W2
        lastT = const.tile([P, DC, BSH], bf)
        for c in range(DC):
            lp = aps.tile([P, BSH], bf, tag="aps")
            nc.tensor.transpose(
                lp[:], lastrows_sb[:, c * P : (c + 1) * P], ident_sb[:]
            )
            nc.vector.tensor_copy(lastT[:, c, :], lp[:])

        qb = const.tile([P, UC, BSH], f32)
        for u in range(UC):
            qp = aps.tile([P, BSH], f32, tag="aps")
            for c in range(DC):
                nc.tensor.matmul(
                    qp[:],
                    w2_sb[:, c, u * P : (u + 1) * P],
                    lastT[:, c, :],
                    start=(c == 0),
                    stop=(c == DC - 1),
                )
            nc.vector.tensor_scalar_add(qb[:, u], qp[:], bsum_sb[:, u : u + 1])

        # e rows: e_big[32s, b, :] = exp(score(b, strip s)); e4 = [4, T]
        e_big = const.tile([P, BSH, TS], bf)
        e4 = const.tile([BSH, T], bf)
        scps = []

        def emit_exp(b):
            # Exp per score strip (ACT cannot read strided partitions)
            for s in range(TN):
                nc.scalar.activation(
                    e_big[32 * s : 32 * s + 1, b, :],
                    scps[b][32 * s : 32 * s + 1, :],
                    Exp,
                )
            # assemble row b of e4 on the vector queue (sync FIFO is busy
            # with bulk loads)
            nc.gpsimd.dma_start(
                e4[b : b + 1, :].rearrange("p (s x) -> p s x", s=TN),
                e_big[0 : 3 * 32 + 1 : 32, b, :],
            )

        # ---- main phase: keys -> tanh -> score strips, batch-major -----
        for b in range(BSH):
            scp = sps.tile([P, TS], f32, tag="sps")
            scps.append(scp)
            for u in range(UC):
                tkts = []
                for h in range(2):
                    kp = kps.tile([P, 2, TS], f32, tag="kps")
                    for j in range(2):
                        t0 = (2 * h + j) * TS
                        for c in range(DC):
                            nc.tensor.matmul(
                                kp[:, j],
                                w1_sb[:, c, u * P : (u + 1) * P],
                                valts[b][:, c, t0 : t0 + TS],
                                start=(c == 0),
                                stop=(c == DC - 1),
                            )
                    tkt = tk_pool.tile([P, 2, TS], bf, tag="tk")
                    nc.scalar.activation(
                        tkt[:], kp[:], Tanh, bias=qb[:, u, b : b + 1]
                    )
                    tkts.append(tkt)
                # 4 score strips col-tiled across PE column groups
                for s in range(TN):
                    nc.tensor.matmul(
                        scp[32 * s : 32 * s + 1, :],
                        v_sb[:, u : u + 1],
                        tkts[s // 2][:, s % 2, :],
                        start=(u == 0),
                        stop=(u == UC - 1),
                        tile_position=(0, 32 * s),
                        skip_group_check=True,
                    )
                if u == 0 and b > 0:
                    emit_exp(b - 1)
        emit_exp(BSH - 1)

        # ---- tail: transpose e4 chunks + col-tiled weighted sum --------
        wp = sps.tile([P, D], f32, tag="sps")
        at_sb = const.tile([P, TK, BSH], bf)
        for k in range(TK):
            at_p = aps.tile([P, BSH], bf, tag="aps")
            nc.tensor.transpose(
                at_p[:], e4[:, k * P : (k + 1) * P], ident_sb[:]
            )
            nc.vector.tensor_copy(at_sb[:, k, :], at_p[:])
            for b in range(BSH):
                nc.tensor.matmul(
                    wp[32 * b : 32 * b + 1, :],
                    at_sb[:, k, b : b + 1],
                    nats[b][:, k, :],
                    start=(k == 0),
                    stop=(k == TK - 1),
                    tile_position=(0, 32 * b),
                    skip_group_check=True,
                )

        # Z_b = sum_t e(b, t) via ones-stationary matmul -> partition 32b
        zr = const.tile([P, 1], f32)
        zrec = const.tile([P, 1], f32)
        for b in range(BSH):
            zk = aps.tile([P, TK], f32, tag="aps")
            nc.tensor.matmul(
                zk[32 * b : 32 * b + 1, :],
                ones_sb[:],
                at_sb[:, :, b],
                start=True,
                stop=True,
                tile_position=(0, 32 * b),
                skip_group_check=True,
            )
            nc.vector.tensor_reduce(
                zr[32 * b : 32 * b + 1, :],
                zk[32 * b : 32 * b + 1, :],
                mybir.AxisListType.X,
                mybir.AluOpType.add,
            )
            nc.vector.reciprocal(
                zrec[32 * b : 32 * b + 1, :], zr[32 * b : 32 * b + 1, :]
            )

        # out rows = wp strip * (1/Z); per-batch (engines need unit
        # partition step), alternating ACT/DVE so the tail ops overlap
        ob = const.tile([P, D], f32)
        for b in range(BSH):
            if b % 2 == 0:
                nc.scalar.mul(
                    ob[32 * b : 32 * b + 1, :],
                    wp[32 * b : 32 * b + 1, :],
                    zrec[32 * b : 32 * b + 1, 0:1],
                )
            else:
                nc.vector.tensor_scalar_mul(
                    ob[32 * b : 32 * b + 1, :],
                    wp[32 * b : 32 * b + 1, :],
                    zrec[32 * b : 32 * b + 1, 0:1],
                )
        nc.gpsimd.dma_start(out_ext.ap()[:, :], ob[0 : 3 * 32 + 1 : 32, :])

    nc.finalize()
    return nc


def _get_graph():
    global _GRAPH
    if _GRAPH is None:
        _GRAPH = _build_graph()
    return _GRAPH


def _make_in_maps(values, W1_w, W1_b, W2_w, W2_b, V_w, V_b):
    vals = np.asarray(values, np.float32)
    w1_bf = np.ascontiguousarray(
        np.asarray(W1_w, np.float32).reshape(DC, P, U).transpose(1, 0, 2)
    ).astype(BF16)
    w2_bf = np.ascontiguousarray(
        np.asarray(W2_w, np.float32).reshape(DC, P, U).transpose(1, 0, 2)
    ).astype(BF16)
    v_bf = np.ascontiguousarray(
        np.asarray(V_w, np.float32).reshape(UC, P).T
    ).astype(BF16)
    bsum = np.ascontiguousarray(
        (np.asarray(W1_b, np.float32) + np.asarray(W2_b, np.float32))
        .reshape(UC, P)
        .T
    )
    ones = np.ones((P, 1), dtype=BF16)
    ident = np.eye(BSH, dtype=BF16)

    in_maps = []
    for core in range(NCORES):
        sl = vals[core * BSH : (core + 1) * BSH]  # [BSH, T, D] f32
        # valt[b, r, c, t] = v[b, t, 128c + r]
        valt = np.ascontiguousarray(
            sl.reshape(BSH, T, DC, P).transpose(0, 3, 2, 1)
        ).astype(BF16)
        # nat[b, r, k, d] = v[b, 128k + r, d]
        nat = np.ascontiguousarray(
            sl.reshape(BSH, TK, P, D).transpose(0, 2, 1, 3)
        ).astype(BF16)
        lastrows = np.ascontiguousarray(sl[:, T - 1, :]).astype(BF16)
        in_maps.append(
            {
                "valt": valt,
                "nat": nat,
                "w1": w1_bf,
                "w2": w2_bf,
                "vw": v_bf,
                "bsum": bsum,
                "ones": ones,
                "lastrows": lastrows,
                "ident": ident,
            }
        )
    return in_maps


def run(inputs, trace=False, **kw):
    """Build + run on 8 cores; returns (full_output, BassKernelResults)."""
    nc = _get_graph()
    in_maps = _make_in_maps(**inputs)
    res = run_bass_kernel_spmd(
        nc, in_maps, core_ids=list(range(NCORES)), trace=trace, **kw
    )
    out = np.concatenate([np.asarray(r["out"]) for r in res.results], axis=0)
    return out.astype(np.float32), res


def kernel(**inputs) -> np.ndarray:
    out, _ = run(inputs)
    return out


# revision 7
# speedup vs baseline: 1.1968x; 1.1692x over previous
"""AdditiveAttention Trainium2 kernel (8 NeuronCores, data-parallel over batch).

Reference computation (B=32, T=2048, D=U=512, fp32):
    query = values[:, -1] @ W2_w + W2_b                     # [B, U]
    keys  = values @ W1_w + W1_b                            # [B, T, U]
    score = tanh(keys + query[:, None, :]) @ V_w + V_b      # [B, T, 1]
    attn  = softmax(score, axis=1)
    out   = sum(attn * values, axis=1)                      # [B, D]

Sharding: data-parallel over B (4 batches per core), weights replicated,
no collectives.  bf16 matmuls (fp32 PSUM accumulate); rel-err ~3e-3.

Design notes (v3):
  - values pre-transposed on the HOST into valt [b, r, c, t] and
    pre-tiled natural nat [b, r, k, d]: every DMA is a plain copy with
    16KB-contiguous per-partition lines (no xbar transpose mode)
  - keys: per (b, u, half) 8 matmuls into kp [P, 2, TS], banks
    alternated (j inner) so consecutive matmuls hit different PSUM banks
  - score: 4 strips col-tiled at PSUM partitions 32s; the group for
    (b, u) is emitted AFTER the next keys half-block so the PE never
    stalls waiting for tanh (in-order engine)
  - softmax: Exp per strip with accum_out -> zpart[32s, b]; Z_b folds
    to partition 32b via a ones-stationary 1-col matmul; 1/Z lands at
    the same partition as the wsum strip so the final scale is one op
  - e rows assembled into e4 [4, T] by one SBUF->SBUF DMA per batch
  - tail: 16 e4-chunk transposes back-to-back, then 16 col-tiled wsum
    groups (4 batches concurrently, ~455ns per 4x512-col group)
"""

from contextlib import ExitStack

import numpy as np
import ml_dtypes

import concourse.bass as bass
import concourse.tile as tile
from concourse import bacc, mybir
from concourse.bass_utils import run_bass_kernel_spmd

BF16 = ml_dtypes.bfloat16

B, T, D, U = 32, 2048, 512, 512
NCORES = 8
BSH = B // NCORES          # 4 batches per core
P = 128
DC = D // P                # 4 chunks of D
UC = U // P                # 4 chunks of U
TS = 512                   # score strip / T tile
TN = T // TS               # 4 strips
TK = T // P                # 16 chunks of T for the weighted sum

_GRAPH = None


def _build_graph():
    nc = bacc.Bacc("TRN2", target_bir_lowering=False, debug=False)
    bf = mybir.dt.bfloat16
    f32 = mybir.dt.float32

    valt = nc.declare_dram_parameter("valt", [BSH, P, DC, T], bf, isOutput=False)
    nat = nc.declare_dram_parameter("nat", [BSH, P, TK, D], bf, isOutput=False)
    w1 = nc.declare_dram_parameter("w1", [P, DC, U], bf, isOutput=False)
    w2 = nc.declare_dram_parameter("w2", [P, DC, U], bf, isOutput=False)
    vw = nc.declare_dram_parameter("vw", [P, UC], bf, isOutput=False)
    bsum = nc.declare_dram_parameter("bsum", [P, UC], f32, isOutput=False)
    ones = nc.declare_dram_parameter("ones", [P, 1], f32, isOutput=False)
    lastrows = nc.declare_dram_parameter("lastrows", [BSH, D], bf, isOutput=False)
    ident = nc.declare_dram_parameter("ident", [BSH, BSH], bf, isOutput=False)
    out_ext = nc.declare_dram_parameter("out", [BSH, D], f32, isOutput=True)

    Tanh = mybir.ActivationFunctionType.Tanh
    Exp = mybir.ActivationFunctionType.Exp

    with tile.TileContext(nc) as tc, ExitStack() as ctx:
        const = ctx.enter_context(tc.tile_pool(name="const", bufs=1))
        valt_pool = ctx.enter_context(tc.tile_pool(name="valt", bufs=BSH))
        nat_pool = ctx.enter_context(tc.tile_pool(name="nat", bufs=BSH))
        tk_pool = ctx.enter_context(tc.tile_pool(name="tk", bufs=3))
        kps = ctx.enter_context(tc.tile_pool(name="kps", bufs=2, space="PSUM"))
        sps = ctx.enter_context(tc.tile_pool(name="sps", bufs=2, space="PSUM"))
        aps = ctx.enter_context(tc.tile_pool(name="aps", bufs=2, space="PSUM"))

        # ---- bulk loads: plain copies, big contiguous descriptors ------
        ident_sb = const.tile([BSH, BSH], bf)
        nc.sync.dma_start(ident_sb[:], ident.ap())
        lastrows_sb = const.tile([BSH, D], bf)
        nc.sync.dma_start(lastrows_sb[:], lastrows.ap())
        bsum_sb = const.tile([P, UC], f32)
        nc.sync.dma_start(bsum_sb[:], bsum.ap())
        v_sb = const.tile([P, UC], bf)
        nc.sync.dma_start(v_sb[:], vw.ap())
        ones_sb = const.tile([P, 1], f32)
        nc.sync.dma_start(ones_sb[:], ones.ap())
        w2_sb = const.tile([P, DC, U], bf)
        nc.sync.dma_start(w2_sb[:], w2.ap())
        w1_sb = const.tile([P, DC, U], bf)
        nc.sync.dma_start(w1_sb[:], w1.ap())

        valts = []
        for b in range(BSH):
            valt_b = valt_pool.tile([P, DC, T], bf, tag="valt")
            nsplit = 4 if b == 0 else 1
            step = T // nsplit
            for i in range(nsplit):
                nc.sync.dma_start(
                    valt_b[:, :, i * step : (i + 1) * step],
                    valt.ap()[b, :, :, i * step : (i + 1) * step],
                )
            valts.append(valt_b)
        nats = []
        for b in range(BSH):
            nat_b = nat_pool.tile([P, TK, D], bf, tag="nat")
            nc.sync.dma_start(nat_b[:], nat.ap()[b])
            nats.append(nat_b)

        # ---- query: lastrows -> lastT via PE transpose; q = lastT.T@W2
        lastT = const.tile([P, DC, BSH], bf)
        for c in range(DC):
            lp = aps.tile([P, BSH], bf, tag="aps")
            nc.tensor.transpose(
                lp[:], lastrows_sb[:, c * P : (c + 1) * P], ident_sb[:]
            )
            nc.vector.tensor_copy(lastT[:, c, :], lp[:])

        qb = const.tile([P, UC, BSH], f32)
        for u in range(UC):
            qp = aps.tile([P, BSH], f32, tag="aps")
            for c in range(DC):
                nc.tensor.matmul(
                    qp[:],
                    w2_sb[:, c, u * P : (u + 1) * P],
                    lastT[:, c, :],
                    start=(c == 0),
                    stop=(c == DC - 1),
                )
            nc.vector.tensor_scalar_add(qb[:, u], qp[:], bsum_sb[:, u : u + 1])

        # softmax state: e rows + per-strip Z partials + 1/Z at part 32b
        e_big = const.tile([P, BSH, TS], bf)
        e4 = const.tile([BSH, T], bf)
        zpart = const.tile([P, BSH], f32)
        nc.scalar.memzero(zpart[:])
        zrec = const.tile([P, 1], f32)
        scps = []
        tkts = {}

        def emit_score(b, u):
            # 4 strips col-tiled across PE column groups; emitted one
            # keys half-block late so tanh(h1) is already done
            for s in range(TN):
                nc.tensor.matmul(
                    scps[b][32 * s : 32 * s + 1, :],
                    v_sb[:, u : u + 1],
                    tkts[(u, s // 2)][:, s % 2, :],
                    start=(u == 0),
                    stop=(u == UC - 1),
                    tile_position=(0, 32 * s),
                    skip_group_check=True,
                )

        def emit_exp(b):
            # Exp per strip (ACT needs unit partition step); accum_out
            # collects the strip total for Z
            for s in range(TN):
                nc.scalar.activation(
                    e_big[32 * s : 32 * s + 1, b, :],
                    scps[b][32 * s : 32 * s + 1, :],
                    Exp,
                    accum_out=zpart[32 * s : 32 * s + 1, b : b + 1],
                )
            # Z_b = sum over partitions of zpart[:, b] -> partition 32b
            zq = aps.tile([P, 1], f32, tag="aps")
            nc.tensor.matmul(
                zq[32 * b : 32 * b + 1, :],
                ones_sb[:],
                zpart[:, b : b + 1],
                start=True,
                stop=True,
                tile_position=(0, 32 * b),
                skip_group_check=True,
            )
            nc.vector.reciprocal(
                zrec[32 * b : 32 * b + 1, :], zq[32 * b : 32 * b + 1, :]
            )
            # assemble row b of e4 (sync FIFO is past the bulk loads)
            nc.sync.dma_start(
                e4[b : b + 1, :].rearrange("p (s x) -> p s x", s=TN),
                e_big[0 : 3 * 32 + 1 : 32, b, :],
            )

        def emit_keys_half(b, u, h):
            kp = kps.tile([P, 2, TS], f32, tag="kps")
            for c in range(DC):
                for j in range(2):
                    nc.tensor.matmul(
                        kp[:, j],
                        w1_sb[:, c, u * P : (u + 1) * P],
                        valts[b][:, c, (2 * h + j) * TS : (2 * h + j + 1) * TS],
                        start=(c == 0),
                        stop=(c == DC - 1),
                    )
            tkt = tk_pool.tile([P, 2, TS], bf, tag="tk")
            nc.scalar.activation(tkt[:], kp[:], Tanh, bias=qb[:, u, b : b + 1])
            tkts[(u, h)] = tkt

        # ---- main phase: keys -> tanh -> (delayed) score, batch-major --
        pending = None          # (b, u) whose score group is not yet out
        for b in range(BSH):
            scp = sps.tile([P, TS], f32, tag="sps")
            scps.append(scp)
            for u in range(UC):
                emit_keys_half(b, u, 0)
                if pending is not None:
                    emit_score(*pending)
                    if pending[1] == UC - 1:
                        emit_exp(pending[0])
                    pending = None
                emit_keys_half(b, u, 1)
                pending = (b, u)
        emit_score(*pending)
        emit_exp(pending[0])

        # ---- tail: all e4-chunk transposes, then col-tiled wsum --------
        wp = sps.tile([P, D], f32, tag="sps")
        at_sb = const.tile([P, TK, BSH], bf)
        at_ps = []
        for k in range(TK):
            at_p = aps.tile([P, BSH], bf, tag="aps")
            nc.tensor.transpose(
                at_p[:], e4[:, k * P : (k + 1) * P], ident_sb[:]
            )
            nc.vector.tensor_copy(at_sb[:, k, :], at_p[:])
        for k in range(TK):
            for b in range(BSH):
                nc.tensor.matmul(
                    wp[32 * b : 32 * b + 1, :],
                    at_sb[:, k, b : b + 1],
                    nats[b][:, k, :],
                    start=(k == 0),
                    stop=(k == TK - 1),
                    tile_position=(0, 32 * b),
                    skip_group_check=True,
                )

        # out rows = wp strip * (1/Z); per-batch ops (unit partition
        # step), alternating ACT/DVE so they overlap
        ob = const.tile([P, D], f32)
        for b in range(BSH):
            if b % 2 == 0:
                nc.scalar.mul(
                    ob[32 * b : 32 * b + 1, :],
                    wp[32 * b : 32 * b + 1, :],
                    zrec[32 * b : 32 * b + 1, 0:1],
                )
            else:
                nc.vector.tensor_scalar_mul(
                    ob[32 * b : 32 * b + 1, :],
                    wp[32 * b : 32 * b + 1, :],
                    zrec[32 * b : 32 * b + 1, 0:1],
                )
        nc.sync.dma_start(out_ext.ap()[:, :], ob[0 : 3 * 32 + 1 : 32, :])

    nc.finalize()
    return nc


def _get_graph():
    global _GRAPH
    if _GRAPH is None:
        _GRAPH = _build_graph()
    return _GRAPH


def _make_in_maps(values, W1_w, W1_b, W2_w, W2_b, V_w, V_b):
    vals = np.asarray(values, np.float32)
    w1_bf = np.ascontiguousarray(
        np.asarray(W1_w, np.float32).reshape(DC, P, U).transpose(1, 0, 2)
    ).astype(BF16)
    w2_bf = np.ascontiguousarray(
        np.asarray(W2_w, np.float32).reshape(DC, P, U).transpose(1, 0, 2)
    ).astype(BF16)
    v_bf = np.ascontiguousarray(
        np.asarray(V_w, np.float32).reshape(UC, P).T
    ).astype(BF16)
    bsum = np.ascontiguousarray(
        (np.asarray(W1_b, np.float32) + np.asarray(W2_b, np.float32))
        .reshape(UC, P)
        .T
    )
    ones = np.ones((P, 1), dtype=np.float32)
    ident = np.eye(BSH, dtype=BF16)

    in_maps = []
    for core in range(NCORES):
        sl = vals[core * BSH : (core + 1) * BSH]  # [BSH, T, D] f32
        # valt[b, r, c, t] = v[b, t, 128c + r]
        valt = np.ascontiguousarray(
            sl.reshape(BSH, T, DC, P).transpose(0, 3, 2, 1)
        ).astype(BF16)
        # nat[b, r, k, d] = v[b, 128k + r, d]
        nat = np.ascontiguousarray(
            sl.reshape(BSH, TK, P, D).transpose(0, 2, 1, 3)
        ).astype(BF16)
        lastrows = np.ascontiguousarray(sl[:, T - 1, :]).astype(BF16)
        in_maps.append(
            {
                "valt": valt,
                "nat": nat,
                "w1": w1_bf,
                "w2": w2_bf,
                "vw": v_bf,
                "bsum": bsum,
                "ones": ones,
                "lastrows": lastrows,
                "ident": ident,
            }
        )
    return in_maps


def run(inputs, trace=False, **kw):
    """Build + run on 8 cores; returns (full_output, BassKernelResults)."""
    nc = _get_graph()
    in_maps = _make_in_maps(**inputs)
    res = run_bass_kernel_spmd(
        nc, in_maps, core_ids=list(range(NCORES)), trace=trace, **kw
    )
    out = np.concatenate([np.asarray(r["out"]) for r in res.results], axis=0)
    return out.astype(np.float32), res


def kernel(**inputs) -> np.ndarray:
    out, _ = run(inputs)
    return out


# revision 9
# speedup vs baseline: 1.3053x; 1.0907x over previous
"""AdditiveAttention Trainium2 kernel (8 NeuronCores, data-parallel over batch).

Reference computation (B=32, T=2048, D=U=512, fp32):
    query = values[:, -1] @ W2_w + W2_b                     # [B, U]
    keys  = values @ W1_w + W1_b                            # [B, T, U]
    score = tanh(keys + query[:, None, :]) @ V_w + V_b      # [B, T, 1]
    attn  = softmax(score, axis=1)
    out   = sum(attn * values, axis=1)                      # [B, D]

Sharding: data-parallel over B (4 batches per core), weights replicated,
no collectives.  bf16 matmuls (fp32 PSUM accumulate); rel-err ~3e-3.

Design notes (v3):
  - values pre-transposed on the HOST into valt [b, r, c, t] and
    pre-tiled natural nat [b, r, k, d]: every DMA is a plain copy with
    16KB-contiguous per-partition lines (no xbar transpose mode)
  - keys: per (b, u, half) 8 matmuls into kp [P, 2, TS], banks
    alternated (j inner) so consecutive matmuls hit different PSUM banks
  - score: 4 strips col-tiled at PSUM partitions 32s; the group for
    (b, u) is emitted AFTER the next keys half-block so the PE never
    stalls waiting for tanh (in-order engine)
  - softmax: Exp per strip with accum_out -> zpart[32s, b]; Z_b folds
    to partition 32b via a ones-stationary 1-col matmul; 1/Z lands at
    the same partition as the wsum strip so the final scale is one op
  - e rows assembled into e4 [4, T] by one SBUF->SBUF DMA per batch
  - tail: 16 e4-chunk transposes back-to-back, then 16 col-tiled wsum
    groups (4 batches concurrently, ~455ns per 4x512-col group)
"""

from contextlib import ExitStack

import numpy as np
import ml_dtypes

import concourse.bass as bass
import concourse.tile as tile
from concourse import bacc, mybir
from concourse.bass_utils import run_bass_kernel_spmd

BF16 = ml_dtypes.bfloat16

B, T, D, U = 32, 2048, 512, 512
NCORES = 8
BSH = B // NCORES          # 4 batches per core
P = 128
DC = D // P                # 4 chunks of D
UC = U // P                # 4 chunks of U
TS = 512                   # score strip / T tile
TN = T // TS               # 4 strips
TK = T // P                # 16 chunks of T for the weighted sum

_GRAPH = None


def _build_graph():
    nc = bacc.Bacc("TRN2", target_bir_lowering=False, debug=False)
    bf = mybir.dt.bfloat16
    f32 = mybir.dt.float32

    valt = nc.declare_dram_parameter("valt", [BSH, P, DC, T], bf, isOutput=False)
    nat = nc.declare_dram_parameter("nat", [BSH, P, TK, D], bf, isOutput=False)
    w1 = nc.declare_dram_parameter("w1", [P, DC, U], bf, isOutput=False)
    w2 = nc.declare_dram_parameter("w2", [P, DC, U], bf, isOutput=False)
    vw = nc.declare_dram_parameter("vw", [P, UC], bf, isOutput=False)
    bsum = nc.declare_dram_parameter("bsum", [P, UC], f32, isOutput=False)
    ones = nc.declare_dram_parameter("ones", [P, 1], f32, isOutput=False)
    lastrows = nc.declare_dram_parameter("lastrows", [BSH, D], bf, isOutput=False)
    ident = nc.declare_dram_parameter("ident", [BSH, BSH], bf, isOutput=False)
    out_ext = nc.declare_dram_parameter("out", [BSH, D], f32, isOutput=True)

    Tanh = mybir.ActivationFunctionType.Tanh
    Exp = mybir.ActivationFunctionType.Exp

    with tile.TileContext(nc) as tc, ExitStack() as ctx:
        const = ctx.enter_context(tc.tile_pool(name="const", bufs=1))
        valt_pool = ctx.enter_context(tc.tile_pool(name="valt", bufs=BSH))
        nat_pool = ctx.enter_context(tc.tile_pool(name="nat", bufs=BSH))
        tk_pool = ctx.enter_context(tc.tile_pool(name="tk", bufs=3))
        kps = ctx.enter_context(tc.tile_pool(name="kps", bufs=2, space="PSUM"))
        sps = ctx.enter_context(tc.tile_pool(name="sps", bufs=2, space="PSUM"))
        aps = ctx.enter_context(tc.tile_pool(name="aps", bufs=2, space="PSUM"))

        # ---- bulk loads: plain copies, big contiguous descriptors ------
        ident_sb = const.tile([BSH, BSH], bf)
        nc.sync.dma_start(ident_sb[:], ident.ap())
        lastrows_sb = const.tile([BSH, D], bf)
        nc.sync.dma_start(lastrows_sb[:], lastrows.ap())
        bsum_sb = const.tile([P, UC], f32)
        nc.sync.dma_start(bsum_sb[:], bsum.ap())
        v_sb = const.tile([P, UC], bf)
        nc.sync.dma_start(v_sb[:], vw.ap())
        ones_sb = const.tile([P, 1], f32)
        nc.sync.dma_start(ones_sb[:], ones.ap())
        w1_sb = const.tile([P, DC, U], bf)
        nc.sync.dma_start(w1_sb[:], w1.ap())
        w2_sb = const.tile([P, DC, U], bf)

        valts = []
        for b in range(BSH):
            valt_b = valt_pool.tile([P, DC, T], bf, tag="valt")
            nsplit = 4 if b == 0 else 1
            step = T // nsplit
            for i in range(nsplit):
                nc.sync.dma_start(
                    valt_b[:, :, i * step : (i + 1) * step],
                    valt.ap()[b, :, :, i * step : (i + 1) * step],
                )
            valts.append(valt_b)
            if b == 0:
                nc.sync.dma_start(w2_sb[:], w2.ap())
        nats = []
        for b in range(BSH):
            nat_b = nat_pool.tile([P, TK, D], bf, tag="nat")
            nc.sync.dma_start(nat_b[:], nat.ap()[b])
            nats.append(nat_b)

        # ---- query: lastrows -> lastT via PE transpose; q = lastT.T@W2
        lastT = const.tile([P, DC, BSH], bf)
        for c in range(DC):
            lp = aps.tile([P, BSH], bf, tag="aps")
            nc.tensor.transpose(
                lp[:], lastrows_sb[:, c * P : (c + 1) * P], ident_sb[:]
            )
            nc.vector.tensor_copy(lastT[:, c, :], lp[:])

        qb = const.tile([P, UC, BSH], f32)
        for u in range(UC):
            qp = aps.tile([P, BSH], f32, tag="aps")
            for c in range(DC):
                nc.tensor.matmul(
                    qp[:],
                    w2_sb[:, c, u * P : (u + 1) * P],
                    lastT[:, c, :],
                    start=(c == 0),
                    stop=(c == DC - 1),
                )
            nc.vector.tensor_scalar_add(qb[:, u], qp[:], bsum_sb[:, u : u + 1])

        # softmax state: e rows + per-strip Z partials + 1/Z at part 32b
        e_big = const.tile([P, BSH, TS], bf)
        e4 = const.tile([BSH, T], bf)
        zpart = const.tile([P, BSH], f32)
        nc.scalar.memzero(zpart[:])
        zrec = const.tile([P, 1], f32)
        scps = []
        tkts = {}

        def emit_score(b, u):
            # 4 strips col-tiled across PE column groups; emitted one
            # keys half-block late so tanh(h1) is already done
            for s in range(TN):
                nc.tensor.matmul(
                    scps[b][32 * s : 32 * s + 1, :],
                    v_sb[:, u : u + 1],
                    tkts[(u, s // 2)][:, s % 2, :],
                    start=(u == 0),
                    stop=(u == UC - 1),
                    tile_position=(0, 32 * s),
                    skip_group_check=True,
                )

        def emit_exp(b):
            # one Exp over the whole bank: off-strip rows hold stale but
            # bounded scores, are never read, and the Z mask zeroes them
            nc.scalar.activation(
                e_big[:, b, :],
                scps[b][:, :],
                Exp,
                accum_out=zpart[:, b : b + 1],
            )
            # assemble row b of e4 (sync FIFO is past the bulk loads)
            nc.sync.dma_start(
                e4[b : b + 1, :].rearrange("p (s x) -> p s x", s=TN),
                e_big[0 : 3 * 32 + 1 : 32, b, :],
            )

        def emit_z(b):
            # Z_b = mask . zpart[:, b] -> partition 32b (mask keeps only
            # the 4 strip rows); deferred so the PE never waits on Exp
            zq = aps.tile([P, 1], f32, tag="aps")
            nc.tensor.matmul(
                zq[32 * b : 32 * b + 1, :],
                ones_sb[:],
                zpart[:, b : b + 1],
                start=True,
                stop=True,
                tile_position=(0, 32 * b),
                skip_group_check=True,
            )
            nc.vector.reciprocal(
                zrec[32 * b : 32 * b + 1, :], zq[32 * b : 32 * b + 1, :]
            )

        def emit_keys_half(b, u, h):
            kp = kps.tile([P, 2, TS], f32, tag="kps")
            for c in range(DC):
                for j in range(2):
                    nc.tensor.matmul(
                        kp[:, j],
                        w1_sb[:, c, u * P : (u + 1) * P],
                        valts[b][:, c, (2 * h + j) * TS : (2 * h + j + 1) * TS],
                        start=(c == 0),
                        stop=(c == DC - 1),
                    )
            tkt = tk_pool.tile([P, 2, TS], bf, tag="tk")
            nc.scalar.activation(tkt[:], kp[:], Tanh, bias=qb[:, u, b : b + 1])
            tkts[(u, h)] = tkt

        # ---- main phase: keys -> tanh -> (delayed) score, batch-major --
        pending = None          # (b, u) whose score group is not yet out
        for b in range(BSH):
            scp = sps.tile([P, TS], f32, tag="sps")
            scps.append(scp)
            for u in range(UC):
                emit_keys_half(b, u, 0)
                if pending is not None:
                    emit_score(*pending)
                    if pending[1] == UC - 1:
                        emit_exp(pending[0])
                    pending = None
                if u == 1 and b > 0:
                    emit_z(b - 1)
                emit_keys_half(b, u, 1)
                pending = (b, u)
        emit_score(*pending)
        emit_exp(pending[0])
        emit_z(BSH - 1)

        # ---- tail: all e4-chunk transposes, then col-tiled wsum --------
        wp = sps.tile([P, D], f32, tag="sps")
        at_sb = const.tile([P, TK, BSH], bf)
        at_ps = []
        for k in range(TK):
            at_p = aps.tile([P, BSH], bf, tag="aps")
            nc.tensor.transpose(
                at_p[:], e4[:, k * P : (k + 1) * P], ident_sb[:]
            )
            if k % 2 == 0:
                nc.vector.tensor_copy(at_sb[:, k, :], at_p[:])
            else:
                nc.scalar.copy(at_sb[:, k, :], at_p[:])
        for k in range(TK):
            for b in range(BSH):
                nc.tensor.matmul(
                    wp[32 * b : 32 * b + 1, :],
                    at_sb[:, k, b : b + 1],
                    nats[b][:, k, :],
                    start=(k == 0),
                    stop=(k == TK - 1),
                    tile_position=(0, 32 * b),
                    skip_group_check=True,
                )

        # out rows = wp strip * (1/Z); ACT and DVE each get their own
        # output tile so the four scales run pairwise in parallel
        ob_a = const.tile([P, D], f32)
        ob_v = const.tile([P, D], f32)
        for b in range(BSH):
            if b % 2 == 0:
                nc.scalar.mul(
                    ob_a[32 * b : 32 * b + 1, :],
                    wp[32 * b : 32 * b + 1, :],
                    zrec[32 * b : 32 * b + 1, 0:1],
                )
            else:
                nc.vector.tensor_scalar_mul(
                    ob_v[32 * b : 32 * b + 1, :],
                    wp[32 * b : 32 * b + 1, :],
                    zrec[32 * b : 32 * b + 1, 0:1],
                )
        nc.sync.dma_start(out_ext.ap()[0:3:2, :], ob_a[0:128:64, :])
        nc.sync.dma_start(out_ext.ap()[1:4:2, :], ob_v[32:128:64, :])

    nc.finalize()
    return nc


def _get_graph():
    global _GRAPH
    if _GRAPH is None:
        _GRAPH = _build_graph()
    return _GRAPH


def _make_in_maps(values, W1_w, W1_b, W2_w, W2_b, V_w, V_b):
    vals = np.asarray(values, np.float32)
    w1_bf = np.ascontiguousarray(
        np.asarray(W1_w, np.float32).reshape(DC, P, U).transpose(1, 0, 2)
    ).astype(BF16)
    w2_bf = np.ascontiguousarray(
        np.asarray(W2_w, np.float32).reshape(DC, P, U).transpose(1, 0, 2)
    ).astype(BF16)
    v_bf = np.ascontiguousarray(
        np.asarray(V_w, np.float32).reshape(UC, P).T
    ).astype(BF16)
    bsum = np.ascontiguousarray(
        (np.asarray(W1_b, np.float32) + np.asarray(W2_b, np.float32))
        .reshape(UC, P)
        .T
    )
    ones = np.zeros((P, 1), dtype=np.float32)
    ones[0 : 3 * 32 + 1 : 32] = 1.0
    ident = np.eye(BSH, dtype=BF16)

    in_maps = []
    for core in range(NCORES):
        sl = vals[core * BSH : (core + 1) * BSH]  # [BSH, T, D] f32
        # valt[b, r, c, t] = v[b, t, 128c + r]
        valt = np.ascontiguousarray(
            sl.reshape(BSH, T, DC, P).transpose(0, 3, 2, 1)
        ).astype(BF16)
        # nat[b, r, k, d] = v[b, 128k + r, d]
        nat = np.ascontiguousarray(
            sl.reshape(BSH, TK, P, D).transpose(0, 2, 1, 3)
        ).astype(BF16)
        lastrows = np.ascontiguousarray(sl[:, T - 1, :]).astype(BF16)
        in_maps.append(
            {
                "valt": valt,
                "nat": nat,
                "w1": w1_bf,
                "w2": w2_bf,
                "vw": v_bf,
                "bsum": bsum,
                "ones": ones,
                "lastrows": lastrows,
                "ident": ident,
            }
        )
    return in_maps


def run(inputs, trace=False, **kw):
    """Build + run on 8 cores; returns (full_output, BassKernelResults)."""
    nc = _get_graph()
    in_maps = _make_in_maps(**inputs)
    res = run_bass_kernel_spmd(
        nc, in_maps, core_ids=list(range(NCORES)), trace=trace, **kw
    )
    out = np.concatenate([np.asarray(r["out"]) for r in res.results], axis=0)
    return out.astype(np.float32), res


def kernel(**inputs) -> np.ndarray:
    out, _ = run(inputs)
    return out


# revision 11
# speedup vs baseline: 1.3414x; 1.0276x over previous
"""AdditiveAttention Trainium2 kernel (8 NeuronCores, data-parallel over batch).

Reference computation (B=32, T=2048, D=U=512, fp32):
    query = values[:, -1] @ W2_w + W2_b                     # [B, U]
    keys  = values @ W1_w + W1_b                            # [B, T, U]
    score = tanh(keys + query[:, None, :]) @ V_w + V_b      # [B, T, 1]
    attn  = softmax(score, axis=1)
    out   = sum(attn * values, axis=1)                      # [B, D]

Sharding: data-parallel over B (4 batches per core), weights replicated,
no collectives.  bf16 matmuls (fp32 PSUM accumulate); rel-err ~3e-3.

Design notes (v3):
  - values pre-transposed on the HOST into valt [b, r, c, t] and
    pre-tiled natural nat [b, r, k, d]: every DMA is a plain copy with
    16KB-contiguous per-partition lines (no xbar transpose mode)
  - keys: per (b, u, half) 8 matmuls into kp [P, 2, TS], banks
    alternated (j inner) so consecutive matmuls hit different PSUM banks
  - score: 4 strips col-tiled at PSUM partitions 32s; the group for
    (b, u) is emitted AFTER the next keys half-block so the PE never
    stalls waiting for tanh (in-order engine)
  - softmax: Exp per strip with accum_out -> zpart[32s, b]; Z_b folds
    to partition 32b via a ones-stationary 1-col matmul; 1/Z lands at
    the same partition as the wsum strip so the final scale is one op
  - e rows assembled into e4 [4, T] by one SBUF->SBUF DMA per batch
  - tail: 16 e4-chunk transposes back-to-back, then 16 col-tiled wsum
    groups (4 batches concurrently, ~455ns per 4x512-col group)
"""

from contextlib import ExitStack

import numpy as np
import ml_dtypes

import concourse.bass as bass
import concourse.tile as tile
from concourse import bacc, mybir
from concourse.bass_utils import run_bass_kernel_spmd

BF16 = ml_dtypes.bfloat16

B, T, D, U = 32, 2048, 512, 512
NCORES = 8
BSH = B // NCORES          # 4 batches per core
P = 128
DC = D // P                # 4 chunks of D
UC = U // P                # 4 chunks of U
TS = 512                   # score strip / T tile
TN = T // TS               # 4 strips
TK = T // P                # 16 chunks of T for the weighted sum

_GRAPH = None


def _build_graph():
    nc = bacc.Bacc("TRN2", target_bir_lowering=False, debug=False)
    bf = mybir.dt.bfloat16
    f32 = mybir.dt.float32

    valt = nc.declare_dram_parameter("valt", [BSH, P, DC, T], bf, isOutput=False)
    nat = nc.declare_dram_parameter("nat", [BSH, P, TK, D], bf, isOutput=False)
    w1 = nc.declare_dram_parameter("w1", [P, DC, U], bf, isOutput=False)
    w2 = nc.declare_dram_parameter("w2", [P, DC, U], bf, isOutput=False)
    vw = nc.declare_dram_parameter("vw", [P, UC], bf, isOutput=False)
    bsum = nc.declare_dram_parameter("bsum", [P, UC], f32, isOutput=False)
    ones = nc.declare_dram_parameter("ones", [P, 1], f32, isOutput=False)
    lastrows = nc.declare_dram_parameter("lastrows", [BSH, D], bf, isOutput=False)
    ident = nc.declare_dram_parameter("ident", [BSH, BSH], bf, isOutput=False)
    out_ext = nc.declare_dram_parameter("out", [BSH, D], f32, isOutput=True)

    Tanh = mybir.ActivationFunctionType.Tanh
    Exp = mybir.ActivationFunctionType.Exp

    with tile.TileContext(nc) as tc, ExitStack() as ctx:
        const = ctx.enter_context(tc.tile_pool(name="const", bufs=1))
        valt_pool = ctx.enter_context(tc.tile_pool(name="valt", bufs=BSH))
        nat_pool = ctx.enter_context(tc.tile_pool(name="nat", bufs=BSH))
        tk_pool = ctx.enter_context(tc.tile_pool(name="tk", bufs=3))
        kps = ctx.enter_context(tc.tile_pool(name="kps", bufs=2, space="PSUM"))
        sps = ctx.enter_context(tc.tile_pool(name="sps", bufs=2, space="PSUM"))
        aps = ctx.enter_context(tc.tile_pool(name="aps", bufs=2, space="PSUM"))

        # ---- bulk loads: plain copies, big contiguous descriptors ------
        ident_sb = const.tile([BSH, BSH], bf)
        nc.sync.dma_start(ident_sb[:], ident.ap())
        lastrows_sb = const.tile([BSH, D], bf)
        nc.sync.dma_start(lastrows_sb[:], lastrows.ap())
        bsum_sb = const.tile([P, UC], f32)
        nc.sync.dma_start(bsum_sb[:], bsum.ap())
        v_sb = const.tile([P, UC], bf)
        nc.sync.dma_start(v_sb[:], vw.ap())
        ones_sb = const.tile([P, 1], f32)
        nc.sync.dma_start(ones_sb[:], ones.ap())
        w1_sb = const.tile([P, DC, U], bf)
        nc.sync.dma_start(w1_sb[:], w1.ap())
        w2_sb = const.tile([P, DC, U], bf)

        valts = []
        for b in range(BSH):
            valt_b = valt_pool.tile([P, DC, T], bf, tag="valt")
            nsplit = 4 if b == 0 else 1
            step = T // nsplit
            for i in range(nsplit):
                nc.sync.dma_start(
                    valt_b[:, :, i * step : (i + 1) * step],
                    valt.ap()[b, :, :, i * step : (i + 1) * step],
                )
            valts.append(valt_b)
            if b == 0:
                nc.sync.dma_start(w2_sb[:], w2.ap())
        nats = []
        for b in range(BSH):
            nat_b = nat_pool.tile([P, TK, D], bf, tag="nat")
            nc.sync.dma_start(nat_b[:], nat.ap()[b])
            nats.append(nat_b)

        # ---- query: lastrows -> lastT via PE transpose; q = lastT.T@W2
        lastT = const.tile([P, DC, BSH], bf)
        for c in range(DC):
            lp = aps.tile([P, BSH], bf, tag="aps")
            nc.tensor.transpose(
                lp[:], lastrows_sb[:, c * P : (c + 1) * P], ident_sb[:]
            )
            nc.vector.tensor_copy(lastT[:, c, :], lp[:])

        qb = const.tile([P, UC, BSH], f32)
        for u in range(UC):
            qp = aps.tile([P, BSH], f32, tag="aps")
            for c in range(DC):
                nc.tensor.matmul(
                    qp[:],
                    w2_sb[:, c, u * P : (u + 1) * P],
                    lastT[:, c, :],
                    start=(c == 0),
                    stop=(c == DC - 1),
                )
            nc.vector.tensor_scalar_add(qb[:, u], qp[:], bsum_sb[:, u : u + 1])

        # softmax state: e rows + per-strip Z partials + 1/Z at part 32b
        e_big = const.tile([P, BSH, TS], bf)
        e4 = const.tile([BSH, T], bf)
        zpart = const.tile([P, BSH], f32)
        nc.scalar.memzero(zpart[:])
        zrec = const.tile([P, 1], f32)
        scps = []
        tkts = {}

        def emit_score(b, u):
            # 4 strips col-tiled across PE column groups; emitted one
            # keys half-block late so tanh(h1) is already done
            for s in range(TN):
                nc.tensor.matmul(
                    scps[b][32 * s : 32 * s + 1, :],
                    v_sb[:, u : u + 1],
                    tkts[(u, s // 2)][:, s % 2, :],
                    start=(u == 0),
                    stop=(u == UC - 1),
                    tile_position=(0, 32 * s),
                    skip_group_check=True,
                )

        def emit_exp(b):
            # one Exp over the whole bank: off-strip rows hold stale but
            # bounded scores, are never read, and the Z mask zeroes them
            nc.scalar.activation(
                e_big[:, b, :],
                scps[b][:, :],
                Exp,
                accum_out=zpart[:, b : b + 1],
            )
            # assemble row b of e4 on the scalar queue (same engine as
            # the Exp that produced it; sync stays pure bulk loads)
            nc.scalar.dma_start(
                e4[b : b + 1, :].rearrange("p (s x) -> p s x", s=TN),
                e_big[0 : 3 * 32 + 1 : 32, b, :],
            )

        def emit_z(b):
            # Z_b = mask . zpart[:, b] -> partition 32b (mask keeps only
            # the 4 strip rows); deferred so the PE never waits on Exp
            zq = aps.tile([P, 1], f32, tag="aps")
            nc.tensor.matmul(
                zq[32 * b : 32 * b + 1, :],
                ones_sb[:],
                zpart[:, b : b + 1],
                start=True,
                stop=True,
                tile_position=(0, 32 * b),
                skip_group_check=True,
            )
            nc.vector.reciprocal(
                zrec[32 * b : 32 * b + 1, :], zq[32 * b : 32 * b + 1, :]
            )

        def emit_keys_half(b, u, h):
            kp = kps.tile([P, 2, TS], f32, tag="kps")
            for c in range(DC):
                for j in range(2):
                    nc.tensor.matmul(
                        kp[:, j],
                        w1_sb[:, c, u * P : (u + 1) * P],
                        valts[b][:, c, (2 * h + j) * TS : (2 * h + j + 1) * TS],
                        start=(c == 0),
                        stop=(c == DC - 1),
                    )
            tkt = tk_pool.tile([P, 2, TS], bf, tag="tk")
            nc.scalar.activation(tkt[:], kp[:], Tanh, bias=qb[:, u, b : b + 1])
            tkts[(u, h)] = tkt

        # ---- main phase: keys -> tanh -> (delayed) score, batch-major --
        pending = None          # (b, u) whose score group is not yet out
        for b in range(BSH):
            scp = sps.tile([P, TS], f32, tag="sps")
            # zero the whole bank: the full-bank Exp reads every row, and
            # exp(stale PSUM) can be Inf, which would NaN the masked Z sum
            nc.vector.memset(scp[:], 0.0)
            scps.append(scp)
            for u in range(UC):
                emit_keys_half(b, u, 0)
                if pending is not None:
                    emit_score(*pending)
                    if pending[1] == UC - 1:
                        emit_exp(pending[0])
                    pending = None
                if u == 1 and b > 0:
                    emit_z(b - 1)
                emit_keys_half(b, u, 1)
                pending = (b, u)
        emit_score(*pending)
        emit_exp(pending[0])
        emit_z(BSH - 1)

        # ---- tail: all e4-chunk transposes, then col-tiled wsum --------
        wp = sps.tile([P, D], f32, tag="sps")
        at_sb = const.tile([P, TK, BSH], bf)
        at_ps = []
        for k in range(TK):
            at_p = aps.tile([P, BSH], bf, tag="aps")
            nc.tensor.transpose(
                at_p[:], e4[:, k * P : (k + 1) * P], ident_sb[:]
            )
            if k % 2 == 0:
                nc.vector.tensor_copy(at_sb[:, k, :], at_p[:])
            else:
                nc.scalar.copy(at_sb[:, k, :], at_p[:])
        for k in range(TK - 1, -1, -1):
            for b in range(BSH):
                nc.tensor.matmul(
                    wp[32 * b : 32 * b + 1, :],
                    at_sb[:, k, b : b + 1],
                    nats[b][:, k, :],
                    start=(k == TK - 1),
                    stop=(k == 0),
                    tile_position=(0, 32 * b),
                    skip_group_check=True,
                )

        # out rows = wp strip * (1/Z); ACT and DVE each get their own
        # output tile so the four scales run pairwise in parallel
        ob_a = const.tile([P, D], f32)
        ob_v = const.tile([P, D], f32)
        for b in range(BSH):
            if b % 2 == 0:
                nc.scalar.mul(
                    ob_a[32 * b : 32 * b + 1, :],
                    wp[32 * b : 32 * b + 1, :],
                    zrec[32 * b : 32 * b + 1, 0:1],
                )
            else:
                nc.vector.tensor_scalar_mul(
                    ob_v[32 * b : 32 * b + 1, :],
                    wp[32 * b : 32 * b + 1, :],
                    zrec[32 * b : 32 * b + 1, 0:1],
                )
        nc.sync.dma_start(out_ext.ap()[0:3:2, :], ob_a[0:128:64, :])
        nc.scalar.dma_start(out_ext.ap()[1:4:2, :], ob_v[32:128:64, :])

    nc.finalize()
    return nc


def _get_graph():
    global _GRAPH
    if _GRAPH is None:
        _GRAPH = _build_graph()
    return _GRAPH


def _make_in_maps(values, W1_w, W1_b, W2_w, W2_b, V_w, V_b):
    vals = np.asarray(values, np.float32)
    w1_bf = np.ascontiguousarray(
        np.asarray(W1_w, np.float32).reshape(DC, P, U).transpose(1, 0, 2)
    ).astype(BF16)
    w2_bf = np.ascontiguousarray(
        np.asarray(W2_w, np.float32).reshape(DC, P, U).transpose(1, 0, 2)
    ).astype(BF16)
    v_bf = np.ascontiguousarray(
        np.asarray(V_w, np.float32).reshape(UC, P).T
    ).astype(BF16)
    bsum = np.ascontiguousarray(
        (np.asarray(W1_b, np.float32) + np.asarray(W2_b, np.float32))
        .reshape(UC, P)
        .T
    )
    ones = np.zeros((P, 1), dtype=np.float32)
    ones[0 : 3 * 32 + 1 : 32] = 1.0
    ident = np.eye(BSH, dtype=BF16)

    in_maps = []
    for core in range(NCORES):
        sl = vals[core * BSH : (core + 1) * BSH]  # [BSH, T, D] f32
        # valt[b, r, c, t] = v[b, t, 128c + r]
        valt = np.ascontiguousarray(
            sl.reshape(BSH, T, DC, P).transpose(0, 3, 2, 1)
        ).astype(BF16)
        # nat[b, r, k, d] = v[b, 128k + r, d]
        nat = np.ascontiguousarray(
            sl.reshape(BSH, TK, P, D).transpose(0, 2, 1, 3)
        ).astype(BF16)
        lastrows = np.ascontiguousarray(sl[:, T - 1, :]).astype(BF16)
        in_maps.append(
            {
                "valt": valt,
                "nat": nat,
                "w1": w1_bf,
                "w2": w2_bf,
                "vw": v_bf,
                "bsum": bsum,
                "ones": ones,
                "lastrows": lastrows,
                "ident": ident,
            }
        )
    return in_maps


def run(inputs, trace=False, **kw):
    """Build + run on 8 cores; returns (full_output, BassKernelResults)."""
    nc = _get_graph()
    in_maps = _make_in_maps(**inputs)
    res = run_bass_kernel_spmd(
        nc, in_maps, core_ids=list(range(NCORES)), trace=trace, **kw
    )
    out = np.concatenate([np.asarray(r["out"]) for r in res.results], axis=0)
    return out.astype(np.float32), res


def kernel(**inputs) -> np.ndarray:
    out, _ = run(inputs)
    return out
